# revision 18
# baseline (speedup 1.0000x reference)
"""Causal self-attention with RoPE on 8 Trainium2 NeuronCores.

Sharding: batch (4) x head-group (2 groups of 8 heads) -> 8 cores.
Each core computes, for its (batch b, head group g):
  qkv projection (fp16 matmuls, x + weights SBUF-resident, loaded once),
  RoPE (rotate-half via +-1 permutation matmul on PE + fp16 DVE mults),
  causal attention (scores fp16, softmax exp on ACT with folded 1/8 scale
  in full-width calls + post-masking, probs/V in fp16, exact-sum
  normalization via ones-column),
  output projection partial (fp16) interleaved into the attention phase.
Host sums the two head-group partials per batch.
"""
import sys

sys.path.insert(0, "/opt/trn_rl_repo")

import numpy as np

import concourse.bass as bass  # noqa: F401
import concourse.mybir as mybir
import concourse.tile as tile
from concourse import bacc
from concourse.bass_utils import run_bass_kernel_spmd

dt = mybir.dt
F32, F16 = dt.float32, dt.float16
ALU = mybir.AluOpType
EXP = mybir.ActivationFunctionType.Exp

ROPE_BASE = 10000.0


def build_core_program(S=2048, D=1024, HL=8, hd=64):
    """Bass program for one core.

    Emission order: loads; v-proj; qk-proj all 8 j-tiles (with RoPE);
    attention pairs 0,1; pairs 2,3 with out-proj tiles interleaved at
    t-chunk granularity so the PE stays fed while ACT runs exp.
    """
    assert hd == 64
    NP = HL // 2           # 4 head pairs
    DT = D // 128          # 8 contraction tiles
    SC = S // 512          # 4 sequence chunks
    ST = S // 128          # 16 seq tiles
    NJT = 2 * NP           # 8 qk j-tiles
    IC = S // 512          # 4 attention query chunks
    NDC = D // 512         # 2 out-proj column chunks
    scale = hd ** -0.5

    nc = bacc.Bacc("TRN2", target_bir_lowering=False, debug=False)
    xT_d = nc.dram_tensor("xT", [D, S], F16, kind="ExternalInput").ap()
    Wqk_d = nc.dram_tensor("Wqk", [D, NJT * 128], F16, kind="ExternalInput").ap()
    Wv_d = nc.dram_tensor("Wv", [D, HL * hd], F16, kind="ExternalInput").ap()
    Wout_d = nc.dram_tensor("Wout", [HL * hd, D], F16, kind="ExternalInput").ap()
    cos_d = nc.dram_tensor("cosT", [128, S], F16, kind="ExternalInput").ap()
    sin_d = nc.dram_tensor("sinT", [128, S], F16, kind="ExternalInput").ap()
    tri_d = nc.dram_tensor("tri", [128, 128], F16, kind="ExternalInput").ap()
    y_d = nc.dram_tensor("y", [S, D], F32, kind="ExternalOutput").ap()

    with tile.TileContext(nc) as tc:
        with tc.tile_pool(name="persist", bufs=1) as pp, \
             tc.tile_pool(name="q16p", bufs=4) as q16p, \
             tc.tile_pool(name="expp", bufs=6) as expp, \
             tc.tile_pool(name="normp", bufs=4) as normp, \
             tc.tile_pool(name="ystp", bufs=2) as ystp, \
             tc.tile_pool(name="projps", bufs=2, space="PSUM") as projps, \
             tc.tile_pool(name="sps", bufs=2, space="PSUM") as sps, \
             tc.tile_pool(name="pavp", bufs=2, space="PSUM") as pavp:

            # ---------------- persistent SBUF tensors ----------------
            xq = pp.tile([128, DT, S], F16, tag="xq")
            wv = pp.tile([128, DT, HL * hd], F16, tag="wv")
            wqk = pp.tile([128, DT, NJT * 128], F16, tag="wqk")
            wout = pp.tile([128, NP, NDC, 512], F16, tag="wout")
            qkT = [pp.tile([128, S], F16, tag=f"qkT{j}", name=f"qkT{j}")
                   for j in range(NJT)]
            v_sb = pp.tile([128, ST, HL, 66], F16, tag="v_sb")
            outT = [pp.tile([128, S], F16, tag=f"outT{p}", name=f"outT{p}")
                    for p in range(NP)]
            cosT = pp.tile([128, S], F16, tag="cosT")
            sinT = pp.tile([128, S], F16, tag="sinT")
            tri = pp.tile([128, 128], F16, tag="tri")

            for ddt in range(DT):
                nc.sync.dma_start(
                    out=wv[:, ddt, :], in_=Wv_d[ddt * 128:(ddt + 1) * 128, :])
            for sc in range(SC):
                for ddt in range(DT):
                    nc.sync.dma_start(
                        out=xq[:, ddt, sc * 512:(sc + 1) * 512],
                        in_=xT_d[ddt * 128:(ddt + 1) * 128,
                                 sc * 512:(sc + 1) * 512])
            for ddt in range(DT):
                nc.sync.dma_start(
                    out=wqk[:, ddt, :], in_=Wqk_d[ddt * 128:(ddt + 1) * 128, :])
            nc.sync.dma_start(out=cosT[:], in_=cos_d[:])
            nc.sync.dma_start(out=sinT[:], in_=sin_d[:])
            nc.sync.dma_start(out=tri[:], in_=tri_d[:])
            for p in range(NP):
                for dc in range(NDC):
                    nc.sync.dma_start(
                        out=wout[:, p, dc, :],
                        in_=Wout_d[p * 128:(p + 1) * 128,
                                   dc * 512:(dc + 1) * 512])
            nc.vector.memset(v_sb[:, :, :, 64:65], 1.0)

            # ---------------- V projection (copies on ACT) ----------------
            for st in range(ST):
                vps = projps.tile([128, 512], F32, tag="pjps", name="vps")
                for ddt in range(DT):
                    nc.tensor.matmul(
                        vps[:],
                        xq[:, ddt, st * 128:(st + 1) * 128],
                        wv[:, ddt, :], start=(ddt == 0), stop=(ddt == DT - 1))
                nc.scalar.copy(
                    v_sb[:, st, :, 0:64],
                    vps[:].rearrange("p (h c) -> p h c", h=HL))

            # ---------------- QK projection + RoPE (one group) ----------------
            # rotate-half entirely on DVE: rotm[dst] = q16[dst^32]*s2[dst]
            # via 4 partition-offset sub-block mults against a host-side
            # pre-swapped signed-sin table (sinT[src] == s2[src^32]).
            ROT_BLOCKS = [(0, 32), (32, 0), (64, 96), (96, 64)]

            def qk_group(jt, sc):
                ss = slice(sc * 512, (sc + 1) * 512)
                qkps = projps.tile([128, 512], F32, tag="pjps", name="qkps")
                for ddt in range(DT):
                    nc.tensor.matmul(
                        qkps[:], wqk[:, ddt, jt * 128:(jt + 1) * 128],
                        xq[:, ddt, ss],
                        start=(ddt == 0), stop=(ddt == DT - 1))
                q16 = q16p.tile([128, 512], F16, tag="q16", name="q16")
                nc.scalar.copy(q16[:], qkps[:])
                rotm = q16p.tile([128, 512], F16, tag="q16", name="rotm")
                for db, sb in ROT_BLOCKS:
                    nc.vector.tensor_tensor(
                        rotm[db:db + 32, :], q16[sb:sb + 32, :],
                        sinT[sb:sb + 32, ss], ALU.mult)
                nc.vector.tensor_tensor(
                    qkT[jt][:, ss], q16[:], cosT[:, ss], ALU.mult)
                nc.vector.tensor_tensor(
                    qkT[jt][:, ss], qkT[jt][:, ss], rotm[:], ALU.add)

            for jt in range(NJT):
                for sc in range(SC):
                    qk_group(jt, sc)

            # ---------------- attention, software-pipelined ----------------
            # AV matmuls lag one unit behind scores/exp (across chunk and
            # pair boundaries) so the PE FIFO never blocks on a pending exp;
            # each chunk's norm is emitted after the next chunk's first
            # scores, and reads pav via a single [65,512] copy so the PSUM
            # bank frees immediately.
            pend = {"av": None, "norm": None}

            def flush_pend():
                if pend["av"] is not None:
                    pend["av"]()
                    pend["av"] = None
                if pend["norm"] is not None:
                    pend["norm"]()
                    pend["norm"] = None

            def attn_chunk(p, t):
                # t indexes 256-query chunks (0..2*IC-1); one jp unit covers
                # 256 keys x both heads of the pair in a single 2-bank spt
                # tile, so exp runs one full-width [128,1024] call per unit.
                qT, kT = qkT[2 * p], qkT[2 * p + 1]
                qs = slice(t * 256, (t + 1) * 256)
                pav = []   # allocated after flush_pend so the previous
                           # chunk's norm (last pav reader) is emitted first
                njp = t + 1
                prev = None   # (jp, et) awaiting AV emission

                def emit_av(jp, et):
                    for hh in range(2):
                        h = 2 * p + hh
                        for jj in range(2):
                            jt = 2 * jp + jj
                            nc.tensor.matmul(
                                pav[hh][0:65, :],
                                v_sb[:, jt, h, 0:65],
                                et[:, hh, jj, :],
                                start=(jp == 0 and jj == 0),
                                stop=(jp == njp - 1 and jj == 1))

                for jp in range(njp):
                    spt = sps.tile([128, 2, 2, 256], F32, tag="spt",
                                   name="spt")
                    for hh in range(2):
                        hb = 64 * hh
                        for jj in range(2):
                            jt = 2 * jp + jj
                            nc.tensor.matmul(
                                spt[:, hh, jj, :],
                                kT[hb:hb + 64, jt * 128:(jt + 1) * 128],
                                qT[hb:hb + 64, qs],
                                start=True, stop=True)
                    et = expp.tile([128, 2, 2, 256], F16, tag="expp",
                                   name="et")
                    nc.scalar.activation(et[:], spt[:], EXP, scale=scale)
                    if jp == njp - 1:   # diagonal unit
                        for hh in range(2):
                            nc.vector.memset(et[:, hh, 1, 0:128], 0.0)
                            for jj in range(2):
                                nc.vector.tensor_tensor(
                                    et[:, hh, jj, 128 * jj:128 * jj + 128],
                                    et[:, hh, jj, 128 * jj:128 * jj + 128],
                                    tri[:], ALU.mult)
                    if jp == 0:
                        flush_pend()
                        pav.extend(
                            pavp.tile([128, 256], F32, tag="pav", name="pav")
                            for _ in range(2))
                    else:
                        emit_av(*prev)
                    prev = (jp, et)

                jp_l, et_l = prev
                pend["av"] = lambda: emit_av(jp_l, et_l)

                def emit_norm(pav=pav, p=p, qs=qs):
                    oars = []
                    for hh in range(2):
                        oa = normp.tile([65, 256], F32, tag="oars",
                                        name="oars")
                        nc.vector.tensor_copy(oa[:], pav[hh][0:65, :])
                        oars.append(oa)
                    for hh in range(2):
                        srow = normp.tile([1, 256], F32, tag="srow",
                                          name="srow")
                        nc.vector.tensor_copy(srow[:], oars[hh][64:65, :])
                        rstage = normp.tile([1, 256], F32, tag="rst",
                                            name="rst")
                        nc.vector.reciprocal_approx_fast(
                            out=rstage[:], in_=srow[:])
                        brec = normp.tile([64, 256], F32, tag="brec",
                                          name="brec")
                        nc.gpsimd.partition_broadcast(brec[:], rstage[:])
                        nc.vector.tensor_tensor(
                            outT[p][64 * hh:64 * hh + 64, qs],
                            oars[hh][0:64, :], brec[:], ALU.mult)

                pend["norm"] = emit_norm

            # ---------------- out-proj for one 512-chunk of seq ----------------
            def out_chunk(t):
                for st in range(4 * t, 4 * t + 4):
                    yp2 = [projps.tile([128, 512], F32, tag="pjps",
                                       name="yps") for _ in range(NDC)]
                    for pb in range(NP):
                        for dc in range(NDC):
                            nc.tensor.matmul(
                                yp2[dc][:],
                                outT[pb][:, st * 128:(st + 1) * 128],
                                wout[:, pb, dc, :],
                                start=(pb == 0), stop=(pb == NP - 1))
                    yst = ystp.tile([128, NDC * 512], F32, tag="yst",
                                    name="yst")
                    for dc in range(NDC):
                        nc.scalar.copy(
                            yst[:, dc * 512:(dc + 1) * 512], yp2[dc][:])
                    nc.sync.dma_start(
                        out=y_d[st * 128:(st + 1) * 128, :], in_=yst[:])

            for t8 in range(2 * IC):
                attn_chunk(0, t8)
                attn_chunk(1, t8)
            for t8 in range(2 * IC):
                attn_chunk(2, t8)
                attn_chunk(3, t8)
                if t8 >= 3 and t8 % 2 == 1:
                    out_chunk((t8 - 3) // 2)
            flush_pend()
            out_chunk(IC - 1)
    nc.compile()
    return nc


def make_tables(S=2048, hd=64):
    inv_freq = 1.0 / (ROPE_BASE ** (np.arange(0, hd, 2, dtype=np.float64) / hd))
    t = np.arange(S, dtype=np.float64)
    freqs = np.outer(t, inv_freq)                    # [S, 32]
    emb = np.concatenate([freqs, freqs], axis=-1)    # [S, 64]
    cos1 = np.cos(emb).T.astype(np.float32)          # [64, S]
    sin1 = np.sin(emb).T.astype(np.float32)
    cosT = np.concatenate([cos1, cos1], axis=0).astype(np.float16)  # [128, S]
    # signed sin for rotate-half: s2 = [-sin[0:32]; sin[32:64]], stored
    # pre-swapped (T[src] = s2[src^32]) so the kernel's partition-offset
    # sub-block mults read src-aligned rows: T = [sin[32:64]; -sin[0:32]].
    t64 = np.concatenate([sin1[32:64], -sin1[0:32]], axis=0)
    sinT = np.concatenate([t64, t64], axis=0).astype(np.float16)
    tri = np.tril(np.ones((128, 128), np.float32)).T.astype(np.float16)
    # tri[j, i] = 1 iff j <= i  (lower-tri transposed = upper-tri in [j, i])
    return cosT, sinT, tri


def make_core_inputs(x, Wqkv, Wout, b, g, HL=8, hd=64):
    """Host-side shard prep for core (batch b, head group g)."""
    B, S, D = x.shape
    H = D // hd
    heads = list(range(g * HL, (g + 1) * HL))
    Wq = Wqkv[:, 0:D].reshape(D, H, hd)
    Wk = Wqkv[:, D:2 * D].reshape(D, H, hd)
    Wv = Wqkv[:, 2 * D:3 * D].reshape(D, H, hd)
    # Wqk j-tile order: q(h0,h1), k(h0,h1), q(h2,h3), k(h2,h3), ...
    blocks = []
    for p in range(HL // 2):
        h0, h1 = heads[2 * p], heads[2 * p + 1]
        blocks.append(np.concatenate([Wq[:, h0], Wq[:, h1]], axis=1))
        blocks.append(np.concatenate([Wk[:, h0], Wk[:, h1]], axis=1))
    Wqk_host = np.ascontiguousarray(np.concatenate(blocks, axis=1), np.float16)
    Wv_host = np.ascontiguousarray(
        Wv[:, heads].reshape(D, HL * hd), np.float16)
    Wout_host = np.ascontiguousarray(
        Wout[g * HL * hd:(g + 1) * HL * hd, :], np.float16)
    xT = np.ascontiguousarray(x[b].T, np.float16)
    cosT, sinT, tri = make_tables(S, hd)
    return {"xT": xT, "Wqk": Wqk_host, "Wv": Wv_host, "Wout": Wout_host,
            "cosT": cosT, "sinT": sinT, "tri": tri}


_NC_CACHE = {}
TRACE = False          # test-only: capture NTFF profile + exec time
LAST_EXEC_NS = None
LAST_RESULT = None


def _enable_ntff_hook():
    import types
    import trn_agent_boot.trn_boot as tb
    import concourse.bass_utils as bu
    m = types.ModuleType("antenv.axon_hooks")
    _hook = [None]
    m.set_axon_ntff_profile_hook = lambda h: _hook.__setitem__(0, h)
    m.get_axon_ntff_profile_hook = lambda: _hook[0]
    sys.modules["antenv.axon_hooks"] = m
    m.set_axon_ntff_profile_hook(
        tb._ntff_profile_via_ctypes("/opt/axon/libaxon_pjrt.so"))
    bu.upload_artifacts = lambda tmpdir: ""


def kernel(x, Wqkv, Wout):
    global LAST_EXEC_NS, LAST_RESULT
    B, S, D = x.shape
    key = (B, S, D)
    if key not in _NC_CACHE:
        _NC_CACHE[key] = build_core_program(S=S, D=D)
    nc = _NC_CACHE[key]
    in_maps = []
    for core in range(8):
        b, g = core // 2, core % 2
        in_maps.append(make_core_inputs(np.asarray(x), np.asarray(Wqkv),
                                        np.asarray(Wout), b, g))
    kw = {}
    if TRACE:
        _enable_ntff_hook()
        kw = dict(trace=True, trace_cores=[0])
    res = run_bass_kernel_spmd(nc, in_maps, core_ids=list(range(8)), **kw)
    LAST_EXEC_NS = res.exec_time_ns
    LAST_RESULT = res
    y = np.empty((B, S, D), np.float32)
    for b in range(B):
        y[b] = res.results[2 * b]["y"] + res.results[2 * b + 1]["y"]
    return y


# revision 22
# speedup vs baseline: 1.0670x; 1.0670x over previous
"""Causal self-attention with RoPE on 8 Trainium2 NeuronCores.

Sharding: batch (4) x head-group (2 groups of 8 heads) -> 8 cores.
Each core computes, for its (batch b, head group g):
  qkv projection (fp16 matmuls, x + weights SBUF-resident, loaded once),
  RoPE (rotate-half via +-1 permutation matmul on PE + fp16 DVE mults),
  causal attention (scores fp16, softmax exp on ACT with folded 1/8 scale
  in full-width calls + post-masking, probs/V in fp16, exact-sum
  normalization via ones-column),
  output projection partial (fp16) interleaved into the attention phase.
Host sums the two head-group partials per batch.
"""
import sys

sys.path.insert(0, "/opt/trn_rl_repo")

import numpy as np

import concourse.bass as bass  # noqa: F401
import concourse.mybir as mybir
import concourse.tile as tile
from concourse import bacc
from concourse.bass_utils import run_bass_kernel_spmd

dt = mybir.dt
F32, F16 = dt.float32, dt.float16
ALU = mybir.AluOpType
EXP = mybir.ActivationFunctionType.Exp

ROPE_BASE = 10000.0


def build_core_program(S=2048, D=1024, HL=8, hd=64):
    """Bass program for one core.

    Emission order: loads; v-proj; qk-proj all 8 j-tiles (with RoPE);
    attention pairs 0,1; pairs 2,3 with out-proj tiles interleaved at
    t-chunk granularity so the PE stays fed while ACT runs exp.
    """
    assert hd == 64
    NP = HL // 2           # 4 head pairs
    DT = D // 128          # 8 contraction tiles
    SC = S // 512          # 4 sequence chunks
    ST = S // 128          # 16 seq tiles
    NJT = 2 * NP           # 8 qk j-tiles
    IC = S // 512          # 4 attention query chunks
    NDC = D // 512         # 2 out-proj column chunks
    scale = hd ** -0.5

    nc = bacc.Bacc("TRN2", target_bir_lowering=False, debug=False)
    xT_d = nc.dram_tensor("xT", [D, S], F16, kind="ExternalInput").ap()
    Wqk_d = nc.dram_tensor("Wqk", [D, NJT * 128], F16, kind="ExternalInput").ap()
    Wv_d = nc.dram_tensor("Wv", [D, HL * hd], F16, kind="ExternalInput").ap()
    Wout_d = nc.dram_tensor("Wout", [HL * hd, D], F16, kind="ExternalInput").ap()
    cos_d = nc.dram_tensor("cosT", [128, S], F16, kind="ExternalInput").ap()
    sin_d = nc.dram_tensor("sinT", [128, S], F16, kind="ExternalInput").ap()
    tri_d = nc.dram_tensor("tri", [128, 128], F16, kind="ExternalInput").ap()
    rotP_d = nc.dram_tensor("rotP", [128, 128], F16, kind="ExternalInput").ap()
    y_d = nc.dram_tensor("y", [S, D], F32, kind="ExternalOutput").ap()

    with tile.TileContext(nc) as tc:
        with tc.tile_pool(name="persist", bufs=1) as pp, \
             tc.tile_pool(name="q16p", bufs=4) as q16p, \
             tc.tile_pool(name="expp", bufs=6) as expp, \
             tc.tile_pool(name="normp", bufs=4) as normp, \
             tc.tile_pool(name="ystp", bufs=2) as ystp, \
             tc.tile_pool(name="projps", bufs=2, space="PSUM") as projps, \
             tc.tile_pool(name="sps", bufs=2, space="PSUM") as sps, \
             tc.tile_pool(name="pavp", bufs=2, space="PSUM") as pavp:

            # ---------------- persistent SBUF tensors ----------------
            xq = pp.tile([128, DT, S], F16, tag="xq")
            wv = pp.tile([128, DT, HL * hd], F16, tag="wv")
            wqk = pp.tile([128, DT, NJT * 128], F16, tag="wqk")
            wout = pp.tile([128, NP, NDC, 512], F16, tag="wout")
            qkT = [pp.tile([128, S], F16, tag=f"qkT{j}", name=f"qkT{j}")
                   for j in range(NJT)]
            v_sb = pp.tile([128, ST, HL, 66], F16, tag="v_sb")
            outT = [pp.tile([128, S], F16, tag=f"outT{p}", name=f"outT{p}")
                    for p in range(NP)]
            cosT = pp.tile([128, S], F16, tag="cosT")
            sinT = pp.tile([128, S], F16, tag="sinT")
            tri = pp.tile([128, 128], F16, tag="tri")
            rotP = pp.tile([128, 128], F16, tag="rotP")

            for ddt in range(DT):
                nc.sync.dma_start(
                    out=wv[:, ddt, :], in_=Wv_d[ddt * 128:(ddt + 1) * 128, :])
            for sc in range(SC):
                for ddt in range(DT):
                    nc.sync.dma_start(
                        out=xq[:, ddt, sc * 512:(sc + 1) * 512],
                        in_=xT_d[ddt * 128:(ddt + 1) * 128,
                                 sc * 512:(sc + 1) * 512])
            for ddt in range(DT):
                nc.sync.dma_start(
                    out=wqk[:, ddt, :], in_=Wqk_d[ddt * 128:(ddt + 1) * 128, :])
            nc.sync.dma_start(out=cosT[:], in_=cos_d[:])
            nc.sync.dma_start(out=sinT[:], in_=sin_d[:])
            nc.sync.dma_start(out=tri[:], in_=tri_d[:])
            nc.sync.dma_start(out=rotP[:], in_=rotP_d[:])
            for p in range(NP):
                for dc in range(NDC):
                    nc.sync.dma_start(
                        out=wout[:, p, dc, :],
                        in_=Wout_d[p * 128:(p + 1) * 128,
                                   dc * 512:(dc + 1) * 512])
            nc.vector.memset(v_sb[:, :, :, 64:65], 1.0)

            # ---------------- V projection (copies on ACT) ----------------
            for st in range(ST):
                vps = projps.tile([128, 512], F32, tag="pjps", name="vps")
                for ddt in range(DT):
                    nc.tensor.matmul(
                        vps[:],
                        xq[:, ddt, st * 128:(st + 1) * 128],
                        wv[:, ddt, :], start=(ddt == 0), stop=(ddt == DT - 1))
                nc.scalar.copy(
                    v_sb[:, st, :, 0:64],
                    vps[:].rearrange("p (h c) -> p h c", h=HL))

            # ---------------- QK projection + RoPE (one group) ----------------
            # The rot matmul + RoPE vector chain for group g is emitted
            # after group g+1's projection matmuls so the PE never waits on
            # the ACT q16 copy. rot_ps borrows the attention-idle pavp pool.
            pend_rot = [None]

            def qk_group(jt, sc):
                ss = slice(sc * 512, (sc + 1) * 512)
                qkps = projps.tile([128, 512], F32, tag="pjps", name="qkps")
                for ddt in range(DT):
                    nc.tensor.matmul(
                        qkps[:], wqk[:, ddt, jt * 128:(jt + 1) * 128],
                        xq[:, ddt, ss],
                        start=(ddt == 0), stop=(ddt == DT - 1))
                q16 = q16p.tile([128, 512], F16, tag="q16", name="q16")
                nc.scalar.copy(q16[:], qkps[:])
                if pend_rot[0] is not None:
                    pend_rot[0]()

                def rot_part(jt=jt, ss=ss, q16=q16):
                    rot_ps = pavp.tile([128, 512], F32, tag="pav",
                                       name="rotps")
                    nc.tensor.matmul(rot_ps[:], rotP[:], q16[:],
                                     start=True, stop=True)
                    rotm = q16p.tile([128, 512], F16, tag="q16", name="rotm")
                    nc.vector.tensor_tensor(
                        rotm[:], rot_ps[:], sinT[:, ss], ALU.mult)
                    nc.vector.tensor_tensor(
                        qkT[jt][:, ss], q16[:], cosT[:, ss], ALU.mult)
                    nc.vector.tensor_tensor(
                        qkT[jt][:, ss], qkT[jt][:, ss], rotm[:], ALU.add)

                pend_rot[0] = rot_part

            for jt in range(NJT):
                for sc in range(SC):
                    qk_group(jt, sc)
            pend_rot[0]()
            pend_rot[0] = None

            # ---------------- attention, software-pipelined ----------------
            # AV matmuls lag one unit behind scores/exp (across chunk and
            # pair boundaries) so the PE FIFO never blocks on a pending exp;
            # each chunk's norm is emitted after the next chunk's first
            # scores, and reads pav via a single [65,512] copy so the PSUM
            # bank frees immediately.
            pend = {"av": None, "norm": None}

            def flush_pend():
                if pend["av"] is not None:
                    pend["av"]()
                    pend["av"] = None
                if pend["norm"] is not None:
                    pend["norm"]()
                    pend["norm"] = None

            def attn_chunk(p, t):
                # t indexes 256-query chunks (0..2*IC-1); one jp unit covers
                # 256 keys x both heads of the pair in a single 2-bank spt
                # tile, so exp runs one full-width [128,1024] call per unit.
                qT, kT = qkT[2 * p], qkT[2 * p + 1]
                qs = slice(t * 256, (t + 1) * 256)
                pav = []   # allocated after flush_pend so the previous
                           # chunk's norm (last pav reader) is emitted first
                njp = t + 1
                prev = None   # (jp, et) awaiting AV emission

                def emit_av(jp, et):
                    for hh in range(2):
                        h = 2 * p + hh
                        for jj in range(2):
                            jt = 2 * jp + jj
                            nc.tensor.matmul(
                                pav[hh][0:65, :],
                                v_sb[:, jt, h, 0:65],
                                et[:, hh, jj, :],
                                start=(jp == 0 and jj == 0),
                                stop=(jp == njp - 1 and jj == 1))

                for jp in range(njp):
                    spt = sps.tile([128, 2, 2, 256], F32, tag="spt",
                                   name="spt")
                    for hh in range(2):
                        hb = 64 * hh
                        for jj in range(2):
                            jt = 2 * jp + jj
                            nc.tensor.matmul(
                                spt[:, hh, jj, :],
                                kT[hb:hb + 64, jt * 128:(jt + 1) * 128],
                                qT[hb:hb + 64, qs],
                                start=True, stop=True)
                    et = expp.tile([128, 2, 2, 256], F16, tag="expp",
                                   name="et")
                    nc.scalar.activation(et[:], spt[:], EXP, scale=scale)
                    if jp == njp - 1:   # diagonal unit
                        for hh in range(2):
                            nc.vector.memset(et[:, hh, 1, 0:128], 0.0)
                            for jj in range(2):
                                nc.vector.tensor_tensor(
                                    et[:, hh, jj, 128 * jj:128 * jj + 128],
                                    et[:, hh, jj, 128 * jj:128 * jj + 128],
                                    tri[:], ALU.mult)
                    if jp == 0:
                        flush_pend()
                        pav.extend(
                            pavp.tile([128, 256], F32, tag="pav", name="pav")
                            for _ in range(2))
                    else:
                        emit_av(*prev)
                    prev = (jp, et)

                jp_l, et_l = prev
                pend["av"] = lambda: emit_av(jp_l, et_l)

                def emit_norm(pav=pav, p=p, qs=qs):
                    oars = []
                    for hh in range(2):
                        oa = normp.tile([65, 256], F32, tag="oars",
                                        name="oars")
                        nc.vector.tensor_copy(oa[:], pav[hh][0:65, :])
                        oars.append(oa)
                    for hh in range(2):
                        srow = normp.tile([1, 256], F32, tag="srow",
                                          name="srow")
                        nc.vector.tensor_copy(srow[:], oars[hh][64:65, :])
                        rstage = normp.tile([1, 256], F32, tag="rst",
                                            name="rst")
                        nc.vector.reciprocal_approx_fast(
                            out=rstage[:], in_=srow[:])
                        brec = normp.tile([64, 256], F32, tag="brec",
                                          name="brec")
                        nc.gpsimd.partition_broadcast(brec[:], rstage[:])
                        nc.vector.tensor_tensor(
                            outT[p][64 * hh:64 * hh + 64, qs],
                            oars[hh][0:64, :], brec[:], ALU.mult)

                pend["norm"] = emit_norm

            # ---------------- out-proj for one 512-chunk of seq ----------------
            def out_chunk(t):
                for st in range(4 * t, 4 * t + 4):
                    yp2 = [projps.tile([128, 512], F32, tag="pjps",
                                       name="yps") for _ in range(NDC)]
                    for pb in range(NP):
                        for dc in range(NDC):
                            nc.tensor.matmul(
                                yp2[dc][:],
                                outT[pb][:, st * 128:(st + 1) * 128],
                                wout[:, pb, dc, :],
                                start=(pb == 0), stop=(pb == NP - 1))
                    yst = ystp.tile([128, NDC * 512], F32, tag="yst",
                                    name="yst")
                    for dc in range(NDC):
                        nc.vector.tensor_copy(
                            yst[:, dc * 512:(dc + 1) * 512], yp2[dc][:])
                    nc.sync.dma_start(
                        out=y_d[st * 128:(st + 1) * 128, :], in_=yst[:])

            for t8 in range(2 * IC):
                attn_chunk(0, t8)
                attn_chunk(1, t8)
            for t8 in range(2 * IC):
                attn_chunk(2, t8)
                attn_chunk(3, t8)
                if t8 >= 3 and t8 % 2 == 1:
                    out_chunk((t8 - 3) // 2)
            flush_pend()
            out_chunk(IC - 1)
    nc.compile()
    return nc


def make_tables(S=2048, hd=64):
    inv_freq = 1.0 / (ROPE_BASE ** (np.arange(0, hd, 2, dtype=np.float64) / hd))
    t = np.arange(S, dtype=np.float64)
    freqs = np.outer(t, inv_freq)                    # [S, 32]
    emb = np.concatenate([freqs, freqs], axis=-1)    # [S, 64]
    cos1 = np.cos(emb).T.astype(np.float32)          # [64, S]
    sin1 = np.sin(emb).T.astype(np.float32)
    cosT = np.concatenate([cos1, cos1], axis=0).astype(np.float16)  # [128, S]
    sinT = np.concatenate([sin1, sin1], axis=0).astype(np.float16)
    tri = np.tril(np.ones((128, 128), np.float32)).T.astype(np.float16)
    # tri[j, i] = 1 iff j <= i  (lower-tri transposed = upper-tri in [j, i])
    # rotP.T @ q = rotate_half(q) with the sign folded in, per 64-dim head:
    #   out[j] = -q[j+32] for j%64 in [0,32), out[j] = q[j-32] for [32,64)
    rotP = np.zeros((128, 128), np.float16)
    for j in range(128):
        base = (j // 64) * 64
        jj = j % 64
        if jj < 32:
            rotP[base + jj + 32, j] = -1.0
        else:
            rotP[base + jj - 32, j] = 1.0
    return cosT, sinT, tri, rotP


def make_core_inputs(x, Wqkv, Wout, b, g, HL=8, hd=64):
    """Host-side shard prep for core (batch b, head group g)."""
    B, S, D = x.shape
    H = D // hd
    heads = list(range(g * HL, (g + 1) * HL))
    Wq = Wqkv[:, 0:D].reshape(D, H, hd)
    Wk = Wqkv[:, D:2 * D].reshape(D, H, hd)
    Wv = Wqkv[:, 2 * D:3 * D].reshape(D, H, hd)
    # Wqk j-tile order: q(h0,h1), k(h0,h1), q(h2,h3), k(h2,h3), ...
    blocks = []
    for p in range(HL // 2):
        h0, h1 = heads[2 * p], heads[2 * p + 1]
        blocks.append(np.concatenate([Wq[:, h0], Wq[:, h1]], axis=1))
        blocks.append(np.concatenate([Wk[:, h0], Wk[:, h1]], axis=1))
    Wqk_host = np.ascontiguousarray(np.concatenate(blocks, axis=1), np.float16)
    Wv_host = np.ascontiguousarray(
        Wv[:, heads].reshape(D, HL * hd), np.float16)
    Wout_host = np.ascontiguousarray(
        Wout[g * HL * hd:(g + 1) * HL * hd, :], np.float16)
    xT = np.ascontiguousarray(x[b].T, np.float16)
    cosT, sinT, tri, rotP = make_tables(S, hd)
    return {"xT": xT, "Wqk": Wqk_host, "Wv": Wv_host, "Wout": Wout_host,
            "cosT": cosT, "sinT": sinT, "tri": tri, "rotP": rotP}


_NC_CACHE = {}
TRACE = False          # test-only: capture NTFF profile + exec time
LAST_EXEC_NS = None
LAST_RESULT = None


def _enable_ntff_hook():
    import types
    import trn_agent_boot.trn_boot as tb
    import concourse.bass_utils as bu
    m = types.ModuleType("antenv.axon_hooks")
    _hook = [None]
    m.set_axon_ntff_profile_hook = lambda h: _hook.__setitem__(0, h)
    m.get_axon_ntff_profile_hook = lambda: _hook[0]
    sys.modules["antenv.axon_hooks"] = m
    m.set_axon_ntff_profile_hook(
        tb._ntff_profile_via_ctypes("/opt/axon/libaxon_pjrt.so"))
    bu.upload_artifacts = lambda tmpdir: ""


def kernel(x, Wqkv, Wout):
    global LAST_EXEC_NS, LAST_RESULT
    B, S, D = x.shape
    key = (B, S, D)
    if key not in _NC_CACHE:
        _NC_CACHE[key] = build_core_program(S=S, D=D)
    nc = _NC_CACHE[key]
    in_maps = []
    for core in range(8):
        b, g = core // 2, core % 2
        in_maps.append(make_core_inputs(np.asarray(x), np.asarray(Wqkv),
                                        np.asarray(Wout), b, g))
    kw = {}
    if TRACE:
        _enable_ntff_hook()
        kw = dict(trace=True, trace_cores=[0])
    res = run_bass_kernel_spmd(nc, in_maps, core_ids=list(range(8)), **kw)
    LAST_EXEC_NS = res.exec_time_ns
    LAST_RESULT = res
    y = np.empty((B, S, D), np.float32)
    for b in range(B):
        y[b] = res.results[2 * b]["y"] + res.results[2 * b + 1]["y"]
    return y
